# revision 3
# baseline (speedup 1.0000x reference)
"""Trainium2 Bass kernel for nn_DynamicLinearModel.

Math:
    c_t   = Z_t[t] . (zeta/2)
    a     = sigmoid(G)
    theta_0 = 0;  theta_t = a*theta_{t-1} + c_{t-1}
    Y[t]  = theta_t + X_t[t] . eta + c_t
    returns (Y, G, eta, zeta)

Strategy (8 NeuronCores, shard along T):
  - Host packs W = [X^T; Z^T] as [128, T] fp32 (feature dim on partitions)
    and slices per core with a 512-row halo.
  - Device streams 128-column blocks of W; for each block one self-loading
    fp32 matmul with the data block as the *stationary* operand and two tiny
    weight columns [eta; zeta/2], [0; zeta/2] as the moving operand. This
    yields e_t = X.eta + Z.zeta/2 and c_t for 128 timesteps spread across
    all 128 PSUM partitions (layout t = 128*block + partition).
  - The scan theta_t = sum_{m>=1} a^(m-1) c_{t-m} truncates exactly in fp32
    after 512 steps (a = sigmoid(G) < 0.82 for any G < 1.5 makes a^512
    underflow to 0), so theta is a short FIR: 5 accumulating matmuls with
    precomputed 128x128 decay matrices against block-shifted c.
  - Y = theta + e on the vector engine, then DMA out.
"""

import numpy as np

from concourse import bass, tile
import concourse.mybir as mybir
from concourse.bass_utils import run_bass_kernel_spmd
from concourse.vector_clock import ScopedClock
import bass_rust

F32 = mybir.dt.float32

T_FULL = 1048576
N_CORES = 8
P = 128
NFEAT = 64
HALO_BLOCKS = 4          # 512-row halo; a^512 == 0 in fp32 for sigmoid(G), G<1.5
R_CORE = T_FULL // N_CORES           # 131072 rows per core
NBLK_OUT = R_CORE // P               # 1024 output blocks per core
NK = HALO_BLOCKS + 1                 # FIR reaches back 5 blocks incl. current


# --- workaround: walrus in this toolchain rejects >1 sem-wait on a Drain ---
def _patched_drain_and_barrier(self, tick_clock, wait_clock):
    nc = self.nc
    drain_inst = nc.sync.drain()
    wait_clock.add_sem_waits(
        drain_inst.ins, ScopedClock({None: tick_clock.global_clock})
    )
    si = drain_inst.ins.sync_info
    if si is not None and si.on_wait and len(si.on_wait) > 1:
        waits = list(si.on_wait)
        si.on_wait = waits[:1]
        for w in waits[1:]:
            extra = nc.sync.drain()
            if extra.ins.sync_info is None:
                extra.ins.sync_info = bass_rust.SyncInfo(on_wait=[w], on_update=[])
            else:
                extra.ins.sync_info.on_wait = [w]
    nc.all_engine_barrier()
    popped = nc._tile_sem_poison_stack.pop()
    assert popped is self._sem_poison
    nc.clear_and_free_semaphores(list(self.sems.allocated().values()))
    nc.all_engine_barrier()


tile.TileContext._drain_and_barrier = _patched_drain_and_barrier


def _split_multi_waits(nc):
    """Walrus here allows at most ONE sem-wait per instruction. Move extra
    waits onto freshly inserted same-engine NOPs placed right before the
    offending instruction (waits still all execute before it, in order)."""
    f = nc.m.functions[0]
    eng_builder = {
        mybir.EngineType.PE: nc.tensor,
        mybir.EngineType.DVE: nc.vector,
        mybir.EngineType.Activation: nc.scalar,
        mybir.EngineType.Pool: nc.gpsimd,
        mybir.EngineType.SP: nc.sync,
    }
    jobs = []
    for blk in f.blocks:
        for inst in blk.instructions:
            si = inst.sync_info
            if si is not None and si.on_wait and len(si.on_wait) > 1:
                jobs.append((blk.name, inst))
    if not jobs:
        return
    created_names = set()
    per_job = []
    for blk_name, inst in jobs:
        waits = list(inst.sync_info.on_wait)
        inst.sync_info.on_wait = waits[-1:]
        nops = []
        for w in waits[:-1]:
            ni = eng_builder[inst.engine].nop().ins
            ni.sync_info = bass_rust.SyncInfo(on_wait=[w], on_update=[])
            nops.append(ni)
            created_names.add(ni.name)
        per_job.append((blk_name, inst.name, nops))
    for blk in f.blocks:
        lst = [i for i in blk.instructions if i.name not in created_names]
        if len(lst) != len(blk.instructions):
            blk.instructions = lst
    for blk_name, iname, nops in per_job:
        blk = next(b for b in f.blocks if b.name == blk_name)
        lst = blk.instructions
        idx = next(i for i, x in enumerate(lst) if x.name == iname)
        blk.instructions = lst[:idx] + nops + lst[idx:]


def build(nblk_out=NBLK_OUT, halo=HALO_BLOCKS, dma_blocks=64, repeat=1):
    """Build the per-core Bass program.

    nblk_out: output blocks (timesteps/128) per core
    halo:     leading halo blocks (c only; e discarded)
    repeat:   emit the whole body N times (timing harness uses slope)
    """
    nblk = nblk_out + halo
    nc = bass.Bass(trn_type="TRN2")
    wt = nc.dram_tensor("wt", [P, nblk * P], F32, kind="ExternalInput")
    w2 = nc.dram_tensor("w2", [P, 2], F32, kind="ExternalInput")
    th = nc.dram_tensor("th", [P, NK * P], F32, kind="ExternalInput")
    y = nc.dram_tensor("y", [P, nblk_out], F32, kind="ExternalOutput")

    ntiles = (nblk + dma_blocks - 1) // dma_blocks
    # theta phase chunks: rhs of a fp32 matmul is capped at 512 columns
    thchunk = 512
    nth = (nblk_out + thchunk - 1) // thchunk

    with tile.TileContext(nc) as tc:
        with (
            tc.tile_pool(name="const", bufs=1) as constp,
            tc.tile_pool(name="wt", bufs=3) as wtp,
            tc.tile_pool(name="ec", bufs=1) as ecp,
            tc.tile_pool(name="yp", bufs=2) as yp,
            tc.tile_pool(name="pec", bufs=4, space="PSUM") as pecp,
            tc.tile_pool(name="pth", bufs=2, space="PSUM") as pthp,
        ):
            w2_sb = constp.tile([P, 2], F32)
            nc.gpsimd.dma_start(w2_sb[:], w2[:])
            th_sb = constp.tile([P, NK * P], F32)
            nc.gpsimd.dma_start(th_sb[:], th[:])

            e_sb = ecp.tile([P, nblk], F32)
            c2_sb = ecp.tile([P, nblk], F32)

            for _ in range(repeat):
                for t in range(ntiles):
                    b0 = t * dma_blocks
                    nb = min(dma_blocks, nblk - b0)
                    wt_sb = wtp.tile([P, dma_blocks * P], F32, tag="wt")
                    nc.gpsimd.dma_start(
                        wt_sb[:, : nb * P], wt[:, b0 * P : (b0 + nb) * P]
                    )
                    pec = pecp.tile([P, 2 * dma_blocks], F32, tag="pec")
                    for j in range(nb):
                        nc.tensor.matmul(
                            pec[:, 2 * j : 2 * j + 2],
                            wt_sb[:, P * j : P * (j + 1)],
                            w2_sb[:],
                            start=True,
                            stop=True,
                        )
                    # de-interleave [e c e c ...] -> e_sb, c2_sb
                    nc.vector.tensor_copy(
                        e_sb[:, b0 : b0 + nb], pec[:, 0 : 2 * nb : 2]
                    )
                    nc.vector.tensor_copy(
                        c2_sb[:, b0 : b0 + nb], pec[:, 1 : 2 * nb : 2]
                    )

                for h in range(nth):
                    ob = h * thchunk
                    nb = min(thchunk, nblk_out - ob)
                    pth = pthp.tile([P, thchunk], F32, tag="pth")
                    for k in range(NK):
                        nc.tensor.matmul(
                            pth[:, :nb],
                            th_sb[:, P * k : P * (k + 1)],
                            c2_sb[:, halo + ob - k : halo + ob - k + nb],
                            start=(k == 0),
                            stop=(k == NK - 1),
                        )
                    y_sb = yp.tile([P, thchunk], F32, tag="y")
                    nc.vector.tensor_add(
                        y_sb[:, :nb], pth[:, :nb], e_sb[:, halo + ob : halo + ob + nb]
                    )
                    nc.gpsimd.dma_start(y[:, ob : ob + nb], y_sb[:, :nb])
    _split_multi_waits(nc)
    return nc


def host_prep(X_t, Z_t, G, eta, zeta, n_cores=N_CORES, nblk_out=NBLK_OUT,
              halo=HALO_BLOCKS):
    """Build per-core input maps (layout packing + tiny scalar-derived tables)."""
    X_t = np.asarray(X_t, dtype=np.float32)
    Z_t = np.asarray(Z_t, dtype=np.float32)
    eta = np.asarray(eta, dtype=np.float32)
    zeta = np.asarray(zeta, dtype=np.float32)
    g = np.float32(np.asarray(G))

    T = n_cores * nblk_out * P
    # W = [X^T; Z^T]  [128, T]
    W = np.empty((P, T), dtype=np.float32)
    W[:NFEAT] = X_t[:T].T
    W[NFEAT:] = Z_t[:T].T

    # weight columns: col0 -> e = X.eta + Z.zeta/2 ; col1 -> c = Z.zeta/2
    w2 = np.zeros((P, 2), dtype=np.float32)
    w2[:NFEAT, 0] = eta
    w2[NFEAT:, 0] = 0.5 * zeta
    w2[NFEAT:, 1] = 0.5 * zeta

    # decay tables: lhsT[tau', tau] = a^(128k + tau - tau' - 1) for exponent>=0
    a32 = np.float32(1.0) / (np.float32(1.0) + np.exp(-g, dtype=np.float32))
    a = np.float64(a32)
    tau = np.arange(P)
    diff = tau[None, :] - tau[:, None]  # [tau', tau]
    th = np.zeros((P, NK * P), dtype=np.float32)
    for k in range(NK):
        E = 128 * k + diff - 1
        with np.errstate(under="ignore"):
            coeff = np.where(E >= 0, a ** np.maximum(E, 0), 0.0)
        th[:, P * k : P * (k + 1)] = coeff.astype(np.float32)

    halo_cols = halo * P
    rows = nblk_out * P
    in_maps = []
    for c in range(n_cores):
        t0 = c * rows
        if t0 >= halo_cols:
            wtc = W[:, t0 - halo_cols : t0 + rows]
        else:
            pad = halo_cols - t0
            wtc = np.concatenate(
                [np.zeros((P, pad), dtype=np.float32), W[:, : t0 + rows]], axis=1
            )
        in_maps.append({"wt": wtc, "w2": w2, "th": th})
    return in_maps


_BUILD_CACHE = {}


def kernel(X_t, Z_t, G, eta, zeta):
    key = (NBLK_OUT, HALO_BLOCKS)
    if key not in _BUILD_CACHE:
        _BUILD_CACHE[key] = build()
    nc = _BUILD_CACHE[key]
    in_maps = host_prep(X_t, Z_t, G, eta, zeta)
    res = run_bass_kernel_spmd(
        nc, in_maps, core_ids=list(range(N_CORES)), trace=False
    )
    parts = [
        np.ascontiguousarray(res.results[c]["y"].T).reshape(-1)
        for c in range(N_CORES)
    ]
    predicted_Y = np.concatenate(parts).astype(np.float32)
    return (
        predicted_Y,
        np.asarray(G, dtype=np.float32),
        np.asarray(eta, dtype=np.float32),
        np.asarray(zeta, dtype=np.float32),
    )
